# revision 9
# baseline (speedup 1.0000x reference)
"""KgAdapterCrossAttention kernel for 8 trn2 NeuronCores.

Sharding: core = (batch b, query-half qh).  Each core computes attention for
1024 queries of one batch element against all 2048 keys.

Engine plan (per core):
  - PE: projections in float32r (1 cyc/row at ap>=256), attention matmuls in
    bf16.  Scores computed transposed S^T [k, q] per head; A computed as
    A^T = V''^T P^T with ap_size=512 and a ones-column in V'' providing the
    softmax denominator (padded to 80 cols so transposes are xbar-friendly).
  - ACT: exp over 2-bank PSUM tiles [128, 2, 512] writing bf16 P^T to SBUF.
    ACT is the bottleneck engine (~66us of exp); everything else is
    scheduled to keep it saturated from ~6us to the end.
  - DVE: multiplicative align-mask (bf16, 2x mode), projection evictions,
    per-q normalize.
  - GPSIMD: PSUM evictions (V'', A^T, output tiles).
  - DMA xbar: transposes (A^T -> A, anorm -> att) for the first q-block;
    the last q-block uses PE transposes to shorten the drain tail.
  - Startup: need-ordered chunked DMAs; V-proj and the K-proj tail are
    interleaved into the first head's pipeline slots.
  - softmax without max-subtraction (scores ~N(0,1)); attention_mask folded
    into the multiplicative mask on host: align * exp(attn_mask) (exact).
"""

import os
import sys

import numpy as np

try:
    import concourse.bass as bass
except ImportError:
    for _p in ("/opt/trn_rl_repo", os.path.expanduser("~/.axon_site/_ro/trn_rl_repo")):
        if os.path.isdir(_p) and _p not in sys.path:
            sys.path.insert(0, _p)
    import concourse.bass as bass

import ml_dtypes
import concourse.mybir as mybir
import concourse.tile as tile
from concourse import bacc
from concourse.masks import make_identity
from contextlib import ExitStack

F32 = mybir.dt.float32
F32R = mybir.dt.float32r
BF16 = mybir.dt.bfloat16
EXP = mybir.ActivationFunctionType.Exp

P = 128
HID = 256
NHEAD = 4
DHEAD = 64
NQ = 1024  # queries per core
NK = 2048  # keys (full)
QBLK = 512
NQB = NQ // QBLK  # 2
NKT = NK // P  # 16
NG = NKT // 2  # 8 kt-pair groups per (qb, h)
NCT = HID // P  # 2 contraction tiles over hidden
VW = 80  # A^T width: 64 V cols + 1 ones col + 15 zero pad (xbar wants %16)
NQT = QBLK // P  # 4


def build() -> bass.Bass:
    nc = bacc.Bacc()
    xqT = nc.declare_dram_parameter("xqT", [HID, NQ], F32, isOutput=False)
    xkT = nc.declare_dram_parameter("xkT", [HID, NK], F32, isOutput=False)
    mm = nc.declare_dram_parameter("mm", [NK, NQ], BF16, isOutput=False)
    wqT = nc.declare_dram_parameter("wqT", [HID, HID], F32, isOutput=False)
    wkT = nc.declare_dram_parameter("wkT", [HID, HID], F32, isOutput=False)
    wvT = nc.declare_dram_parameter("wvT", [HID, HID], F32, isOutput=False)
    woT = nc.declare_dram_parameter("woT", [HID, HID], BF16, isOutput=False)
    out_d = nc.declare_dram_parameter("out", [NQ, HID], F32, isOutput=True)

    def R(ap):
        return ap.bitcast(F32R)

    with tile.TileContext(nc) as tc, ExitStack() as ctx:
        const = ctx.enter_context(tc.tile_pool(name="const", bufs=1))
        big = ctx.enter_context(tc.tile_pool(name="big", bufs=1))
        ptp = ctx.enter_context(tc.tile_pool(name="ptp", bufs=14))
        atp = ctx.enter_context(tc.tile_pool(name="atp", bufs=2))
        wrk = ctx.enter_context(tc.tile_pool(name="wrk", bufs=2))
        obp = ctx.enter_context(tc.tile_pool(name="obp", bufs=2))
        ps_s = ctx.enter_context(tc.tile_pool(name="ps_s", bufs=2, space="PSUM"))
        ps_a = ctx.enter_context(tc.tile_pool(name="ps_a", bufs=2, space="PSUM"))
        ps_o = ctx.enter_context(tc.tile_pool(name="ps_o", bufs=2, space="PSUM"))

        # ---- DMA emission, ordered by first need ----
        def loadw(name, src, dt=F32, cols=None):
            ts = []
            for t in range(2):
                tl = const.tile([P, HID], dt, tag=f"{name}{t}", name=f"{name}{t}")
                nc.sync.dma_start(out=tl, in_=src[t * P : (t + 1) * P, :])
                ts.append(tl)
            return ts

        wq_sb = loadw("wq", wqT)
        xq_sb = [big.tile([P, NQ], F32, tag=f"xq{t}", name=f"xq{t}") for t in range(2)]
        for t in range(2):  # chunk a: first q-block's columns
            nc.sync.dma_start(out=xq_sb[t][:, 0:QBLK], in_=xqT[t * P : (t + 1) * P, 0:QBLK])
        wk_sb = loadw("wk", wkT)
        xk_sb = [big.tile([P, NK], F32, tag=f"xk{t}", name=f"xk{t}") for t in range(2)]
        for c in range(2):  # chunk a in 512-col pieces: kt 0..3, then 4..7
            for t in range(2):
                nc.sync.dma_start(
                    out=xk_sb[t][:, c * QBLK : (c + 1) * QBLK],
                    in_=xkT[t * P : (t + 1) * P, c * QBLK : (c + 1) * QBLK],
                )

        mm_r = mm.rearrange("(t p) q -> p t q", p=P)
        mk0 = big.tile([P, NKT, QBLK], BF16, tag="mk0", name="mk0")
        nc.sync.dma_start(out=mk0[:, 0:4, :], in_=mm_r[:, 0:4, 0:QBLK])
        wv_sb = loadw("wv", wvT)
        nc.sync.dma_start(out=mk0[:, 4:8, :], in_=mm_r[:, 4:8, 0:QBLK])
        for t in range(2):  # chunk b: kt 8..15
            nc.sync.dma_start(
                out=xk_sb[t][:, 1024:2048], in_=xkT[t * P : (t + 1) * P, 1024:2048]
            )
        nc.sync.dma_start(out=mk0[:, 8:12, :], in_=mm_r[:, 8:12, 0:QBLK])
        nc.sync.dma_start(out=mk0[:, 12:16, :], in_=mm_r[:, 12:16, 0:QBLK])
        for t in range(2):  # xq chunk b (only needed by the second q-block)
            nc.sync.dma_start(
                out=xq_sb[t][:, QBLK:NQ], in_=xqT[t * P : (t + 1) * P, QBLK:NQ]
            )
        wo_sb = loadw("wo", woT, dt=BF16)
        mk1 = big.tile([P, NKT, QBLK], BF16, tag="mk1", name="mk1")
        nc.sync.dma_start(out=mk1, in_=mm_r[:, :, QBLK:NQ])
        mks = [mk0, mk1]

        # identity (bf16) for PE transposes + ACT exp-table preload
        ident = const.tile([P, P], BF16, tag="ident", name="ident")
        make_identity(nc, ident)
        warm = wrk.tile([P, 1], F32, tag="warm", name="warm")
        nc.gpsimd.memset(warm, 0.0)
        nc.scalar.activation(warm, warm, EXP)

        qt_sb = [big.tile([P, NQ], BF16, tag=f"qt{t}", name=f"qt{t}") for t in range(2)]
        kt_sb = [big.tile([P, NK], BF16, tag=f"kt{t}", name=f"kt{t}") for t in range(2)]
        v_sb = [None] * NKT

        # ---- compute helpers ----
        def qproj_chunk(t, c):  # q columns c*QBLK:(c+1)*QBLK
            ps = ps_s.tile([P, 2, QBLK], F32, tag="s", name=f"qproj{t}_{c}")
            qs = slice(c * QBLK, (c + 1) * QBLK)
            for ct in range(NCT):
                nc.tensor.matmul(
                    ps[:, 0, :],
                    lhsT=R(wq_sb[ct][:, t * P : (t + 1) * P]),
                    rhs=R(xq_sb[ct][:, qs]),
                    start=(ct == 0),
                    stop=(ct == NCT - 1),
                )
            nc.vector.tensor_copy(qt_sb[t][:, qs], ps[:, 0, :])

        def kproj_chunk(t, c):  # k columns c*1024:(c+1)*1024
            ps = ps_s.tile([P, 2, QBLK], F32, tag="s", name=f"kproj{t}_{c}")
            for nb in range(2):
                off = c * 1024 + nb * QBLK
                for ct in range(NCT):
                    nc.tensor.matmul(
                        ps[:, nb, :],
                        lhsT=R(wk_sb[ct][:, t * P : (t + 1) * P]),
                        rhs=R(xk_sb[ct][:, off : off + QBLK]),
                        start=(ct == 0),
                        stop=(ct == NCT - 1),
                    )
            nc.vector.tensor_copy(
                kt_sb[t][:, c * 1024 : (c + 1) * 1024],
                ps.rearrange("p a b -> p (a b)"),
            )

        def vproj(kt):
            ps = ps_o.tile([P, HID], F32, tag="o", name=f"vproj{kt}")
            for ct in range(NCT):
                nc.tensor.matmul(
                    ps,
                    lhsT=R(xk_sb[ct][:, kt * P : (kt + 1) * P]),
                    rhs=R(wv_sb[ct]),
                    start=(ct == 0),
                    stop=(ct == NCT - 1),
                )
            tl = big.tile([P, NHEAD, VW], BF16, tag=f"v{kt}", name=f"v{kt}")
            nc.gpsimd.tensor_copy(
                tl[:, :, 0:DHEAD], ps.rearrange("p (h d) -> p h d", h=NHEAD)
            )
            nc.gpsimd.memset(tl[:, :, DHEAD : DHEAD + 1], 1.0)
            nc.gpsimd.memset(tl[:, :, DHEAD + 1 : VW], 0.0)
            v_sb[kt] = tl

        # Q-proj chunk a (both t share one psum tile) and the first 1024 K
        # columns of t0, emitted in 512-col pieces so S-matmuls unblock ASAP.
        ps_q = ps_s.tile([P, 2, QBLK], F32, tag="s", name="qproj_a")
        for t in range(2):
            for ct in range(NCT):
                nc.tensor.matmul(
                    ps_q[:, t, :],
                    lhsT=R(wq_sb[ct][:, t * P : (t + 1) * P]),
                    rhs=R(xq_sb[ct][:, 0:QBLK]),
                    start=(ct == 0),
                    stop=(ct == NCT - 1),
                )
            nc.vector.tensor_copy(qt_sb[t][:, 0:QBLK], ps_q[:, t, :])
        ps_k = ps_s.tile([P, 2, QBLK], F32, tag="s", name="kproj_a")
        for c in range(2):
            for ct in range(NCT):
                nc.tensor.matmul(
                    ps_k[:, c, :],
                    lhsT=R(wk_sb[ct][:, 0:P]),
                    rhs=R(xk_sb[ct][:, c * QBLK : (c + 1) * QBLK]),
                    start=(ct == 0),
                    stop=(ct == NCT - 1),
                )
            nc.vector.tensor_copy(kt_sb[0][:, c * QBLK : (c + 1) * QBLK], ps_k[:, c, :])

        # per-slot extra work woven into the first two heads of qb0
        V_SLOT = {g: [2 * g, 2 * g + 1] for g in range(NG)}  # h0 slots
        K_SLOT_H0 = {1: (0, 1)}  # K-proj chunk (t, c) by slot
        K_SLOT_H1 = {0: (1, 0), 1: (1, 1)}

        # ---- attention: one flat slot stream over all (qb, h) pairs ----
        # Each head's drain (last A pair, A^T eviction, transpose, reciprocal,
        # normalize) is deferred into the NEXT head's slot 1 so the ACT engine
        # always has the next head's scores queued.
        anorms_all = {
            qb: [
                wrk.tile([P, HID], BF16, tag=f"an{qt}", name=f"an{qb}_{qt}")
                for qt in range(NQT)
            ]
            for qb in range(NQB)
        }

        def make_S(qb, h, t, po, pts):
            def emit_S(g):
                ps = ps_s.tile([P, 2, QBLK], F32, tag="s", name=f"s{qb}_{h}_{g}")
                for half in range(2):
                    kt = 2 * g + half
                    nc.tensor.matmul(
                        ps[:, half, :],
                        lhsT=kt_sb[t][po : po + DHEAD, kt * P : (kt + 1) * P],
                        rhs=qt_sb[t][po : po + DHEAD, qb * QBLK : (qb + 1) * QBLK],
                        start=True,
                        stop=True,
                    )
                pt = ptp.tile([P, 2, QBLK], BF16, tag="pt", name=f"p{qb}_{h}_{g}")
                nc.scalar.activation(pt, ps, EXP)
                nc.vector.tensor_mul(pt, pt, mks[qb][:, 2 * g : 2 * g + 2, :])
                pts[g] = pt

            return emit_S

        def make_A(h, ps_acc, pts):
            def emit_A(g):
                for half in range(2):
                    kt = 2 * g + half
                    nc.tensor.matmul(
                        ps_acc,
                        lhsT=v_sb[kt][:, h, :],
                        rhs=pts[g][:, half, :],
                        start=(g == 0 and half == 0),
                        stop=(g == NG - 1 and half == 1),
                    )

            return emit_A

        def emit_drain(qb, h, emit_A):
            emit_A(NG - 2)
            emit_A(NG - 1)
            at = atp.tile([VW, QBLK], BF16, tag="at", name=f"at{qb}_{h}")
            nc.gpsimd.tensor_copy(at, ps_accs[(qb, h)])
            if qb < NQB - 1:
                a_t = wrk.tile([P, NQT, VW], BF16, tag=f"a_t{h}", name=f"a_t{qb}_{h}")
                nc.sync.dma_start_transpose(a_t, at)
            else:
                # tail q-block: PE transposes (lower latency than xbar DMA);
                # lives in the ps_o pool, which is idle between V-proj and the
                # output projections.
                a_t = ps_o.tile([P, NQT, VW], BF16, tag="o", name=f"a_tp{qb}_{h}")
                for qt in range(NQT):
                    nc.tensor.transpose(
                        a_t[:, qt, :], at[:, qt * P : (qt + 1) * P], ident[0:VW, 0:VW]
                    )
            rec = wrk.tile([P, NQT, 1], F32, tag=f"rec{h}", name=f"rec{qb}_{h}")
            nc.vector.reciprocal(rec, a_t[:, :, DHEAD : DHEAD + 1])
            # normalize immediately: frees a_t (PSUM in the tail case)
            for qt in range(NQT):
                nc.vector.tensor_scalar_mul(
                    anorms_all[qb][qt][:, h * DHEAD : (h + 1) * DHEAD],
                    a_t[:, qt, 0:DHEAD],
                    rec[:, qt, :],
                )

        def emit_qtloop(qb):
            for qt in range(NQT):
                anorm = anorms_all[qb][qt]
                if qb < NQB - 1:
                    att = wrk.tile([P, NCT, P], BF16, tag="att", name=f"att{qb}_{qt}")
                    nc.sync.dma_start_transpose(att, anorm)
                else:
                    attp = ps_o.tile([P, NCT, P], BF16, tag="o", name=f"attp{qb}_{qt}")
                    for ct in range(NCT):
                        nc.tensor.transpose(
                            attp[:, ct, :], anorm[:, ct * P : (ct + 1) * P], ident
                        )
                    att = wrk.tile([P, NCT, P], BF16, tag="att", name=f"att{qb}_{qt}")
                    nc.vector.tensor_copy(att, attp)
                ps_out = ps_o.tile([P, HID], F32, tag="o", name=f"o{qb}_{qt}")
                for ct in range(NCT):
                    nc.tensor.matmul(
                        ps_out,
                        lhsT=att[:, ct, :],
                        rhs=wo_sb[ct],
                        start=(ct == 0),
                        stop=(ct == NCT - 1),
                    )
                ob = obp.tile([P, HID], F32, tag="ob", name=f"ob{qb}_{qt}")
                nc.gpsimd.tensor_copy(ob, ps_out)
                q0 = qb * QBLK + qt * P
                nc.sync.dma_start(out=out_d[q0 : q0 + P, :], in_=ob)

        ps_accs = {}
        pending = None  # (qb, h, emit_A) awaiting drain
        for qb in range(NQB):
            for h in range(NHEAD):
                t, po = h // 2, (h % 2) * DHEAD
                ps_accs[(qb, h)] = ps_a.tile([VW, QBLK], F32, tag="a", name=f"a{qb}_{h}")
                pts = [None] * NG
                emit_S = make_S(qb, h, t, po, pts)
                emit_A = make_A(h, ps_accs[(qb, h)], pts)
                for g in range(NG):
                    emit_S(g)
                    if qb == 0 and h == 0:
                        for kt in V_SLOT[g]:
                            vproj(kt)
                        if g in K_SLOT_H0:
                            kproj_chunk(*K_SLOT_H0[g])
                    if qb == 0 and h == 1 and g in K_SLOT_H1:
                        kproj_chunk(*K_SLOT_H1[g])
                    if qb == 0 and h == 3 and g == 3:
                        qproj_chunk(0, 1)
                    if qb == 0 and h == 3 and g == 5:
                        qproj_chunk(1, 1)
                    if g >= 2:
                        emit_A(g - 2)
                    if g == 1 and pending is not None:
                        emit_drain(*pending)
                        if pending[0] != qb:  # finished the previous q-block
                            emit_qtloop(pending[0])
                        pending = None
                pending = (qb, h, emit_A)
        emit_drain(*pending)
        emit_qtloop(NQB - 1)
    nc.compile()
    return nc


_NC_CACHE = {}
_last_in_maps = None


def _get_nc() -> bass.Bass:
    if "nc" not in _NC_CACHE:
        _NC_CACHE["nc"] = build()
    return _NC_CACHE["nc"]


def kernel(q_hidden_states, k_hidden_states, attention_mask, align_mask, Wq, Wk, Wv, Wo):
    from concourse.bass_utils import run_bass_kernel_spmd

    q_hidden_states = np.asarray(q_hidden_states, np.float32)
    k_hidden_states = np.asarray(k_hidden_states, np.float32)
    attention_mask = np.asarray(attention_mask, np.float32)
    align_mask = np.asarray(align_mask)
    B, Q, _ = q_hidden_states.shape
    qh_len = Q // 2  # 1024

    nc = _get_nc()

    wq = np.ascontiguousarray(np.asarray(Wq, np.float32).T) / np.float32(8.0)
    wk = np.ascontiguousarray(np.asarray(Wk, np.float32).T)
    wv = np.ascontiguousarray(np.asarray(Wv, np.float32).T)
    wo = np.ascontiguousarray(np.asarray(Wo, np.float32).T).astype(ml_dtypes.bfloat16)

    use_mask = bool(np.any(attention_mask))

    in_maps = []
    for core in range(8):
        b, qh = divmod(core, 2)
        qsl = slice(qh * qh_len, (qh + 1) * qh_len)
        # multiplicative mask: align * exp(attention_mask)  (exact: the
        # reference adds attention_mask pre-exp and zeroes where align==0)
        mmask = align_mask[b, :, qsl].astype(np.float32)
        if use_mask:
            mmask = mmask * np.exp(
                np.ascontiguousarray(attention_mask[b, 0, qsl, :].T), dtype=np.float32
            )
        m = {
            "xqT": np.ascontiguousarray(q_hidden_states[b, qsl].T),
            "xkT": np.ascontiguousarray(k_hidden_states[b].T),
            "mm": np.ascontiguousarray(mmask.astype(ml_dtypes.bfloat16)),
            "wqT": wq,
            "wkT": wk,
            "wvT": wv,
            "woT": wo,
        }
        in_maps.append(m)

    global _last_in_maps
    _last_in_maps = in_maps
    res = run_bass_kernel_spmd(nc, in_maps, list(range(8))).results
    out = np.empty((B, Q, HID), np.float32)
    for core in range(8):
        b, qh = divmod(core, 2)
        out[b, qh * qh_len : (qh + 1) * qh_len] = res[core]["out"]
    return out


# revision 13
# speedup vs baseline: 1.0340x; 1.0340x over previous
"""KgAdapterCrossAttention kernel for 8 trn2 NeuronCores.

Sharding: core = (batch b, query-half qh).  Each core computes attention for
1024 queries of one batch element against all 2048 keys.

Engine plan (per core):
  - PE: projections in float32r (1 cyc/row at ap>=256), attention matmuls in
    bf16.  Scores computed transposed S^T [k, q] per head; A computed as
    A^T = V''^T P^T with ap_size=512 and a ones-column in V'' providing the
    softmax denominator (padded to 80 cols so transposes are xbar-friendly).
  - ACT: exp over 2-bank PSUM tiles [128, 2, 512] writing bf16 P^T to SBUF.
    ACT is the bottleneck engine (~66us of exp); everything else is
    scheduled to keep it saturated from ~6us to the end.
  - DVE: multiplicative align-mask (bf16, 2x mode), projection evictions,
    per-q normalize.
  - GPSIMD: PSUM evictions (V'', A^T, output tiles).
  - DMA xbar: transposes (A^T -> A, anorm -> att) for the first q-block;
    the last q-block uses PE transposes to shorten the drain tail.
  - Startup: need-ordered chunked DMAs; V-proj and the K-proj tail are
    interleaved into the first head's pipeline slots.
  - softmax without max-subtraction (scores ~N(0,1)); attention_mask folded
    into the multiplicative mask on host: align * exp(attn_mask) (exact).
"""

import os
import sys

import numpy as np

try:
    import concourse.bass as bass
except ImportError:
    for _p in ("/opt/trn_rl_repo", os.path.expanduser("~/.axon_site/_ro/trn_rl_repo")):
        if os.path.isdir(_p) and _p not in sys.path:
            sys.path.insert(0, _p)
    import concourse.bass as bass

import ml_dtypes
import concourse.mybir as mybir
import concourse.tile as tile
from concourse import bacc
from concourse.masks import make_identity
from contextlib import ExitStack

F32 = mybir.dt.float32
F32R = mybir.dt.float32r
BF16 = mybir.dt.bfloat16
EXP = mybir.ActivationFunctionType.Exp

P = 128
HID = 256
NHEAD = 4
DHEAD = 64
NQ = 1024  # queries per core
NK = 2048  # keys (full)
QBLK = 512
NQB = NQ // QBLK  # 2
NKT = NK // P  # 16
NG = NKT // 2  # 8 kt-pair groups per (qb, h)
NCT = HID // P  # 2 contraction tiles over hidden
VW = 80  # A^T width: 64 V cols + 1 ones col + 15 zero pad (xbar wants %16)
NQT = QBLK // P  # 4


def build() -> bass.Bass:
    nc = bacc.Bacc()
    xqT = nc.declare_dram_parameter("xqT", [HID, NQ], F32, isOutput=False)
    xkT = nc.declare_dram_parameter("xkT", [HID, NK], F32, isOutput=False)
    mm = nc.declare_dram_parameter("mm", [NK, NQ], BF16, isOutput=False)
    wqT = nc.declare_dram_parameter("wqT", [HID, HID], F32, isOutput=False)
    wkT = nc.declare_dram_parameter("wkT", [HID, HID], F32, isOutput=False)
    wvT = nc.declare_dram_parameter("wvT", [HID, HID], F32, isOutput=False)
    woT = nc.declare_dram_parameter("woT", [HID, HID], BF16, isOutput=False)
    out_d = nc.declare_dram_parameter("out", [NQ, HID], F32, isOutput=True)

    def R(ap):
        return ap.bitcast(F32R)

    with tile.TileContext(nc) as tc, ExitStack() as ctx:
        const = ctx.enter_context(tc.tile_pool(name="const", bufs=1))
        big = ctx.enter_context(tc.tile_pool(name="big", bufs=1))
        ptp = ctx.enter_context(tc.tile_pool(name="ptp", bufs=14))
        atp = ctx.enter_context(tc.tile_pool(name="atp", bufs=2))
        wrk = ctx.enter_context(tc.tile_pool(name="wrk", bufs=2))
        obp = ctx.enter_context(tc.tile_pool(name="obp", bufs=2))
        ps_s = ctx.enter_context(tc.tile_pool(name="ps_s", bufs=2, space="PSUM"))
        ps_a = ctx.enter_context(tc.tile_pool(name="ps_a", bufs=2, space="PSUM"))
        ps_o = ctx.enter_context(tc.tile_pool(name="ps_o", bufs=2, space="PSUM"))

        # ---- DMA emission, ordered by first need ----
        def loadw(name, src, dt=F32, cols=None):
            ts = []
            for t in range(2):
                tl = const.tile([P, HID], dt, tag=f"{name}{t}", name=f"{name}{t}")
                nc.sync.dma_start(out=tl, in_=src[t * P : (t + 1) * P, :])
                ts.append(tl)
            return ts

        wq_sb = loadw("wq", wqT)
        xq_sb = [big.tile([P, NQ], F32, tag=f"xq{t}", name=f"xq{t}") for t in range(2)]
        for t in range(2):  # chunk a: first q-block's columns
            nc.sync.dma_start(out=xq_sb[t][:, 0:QBLK], in_=xqT[t * P : (t + 1) * P, 0:QBLK])
        wk_sb = loadw("wk", wkT)
        xk_sb = [big.tile([P, NK], F32, tag=f"xk{t}", name=f"xk{t}") for t in range(2)]
        for c in range(2):  # chunk a in 512-col pieces: kt 0..3, then 4..7
            for t in range(2):
                nc.sync.dma_start(
                    out=xk_sb[t][:, c * QBLK : (c + 1) * QBLK],
                    in_=xkT[t * P : (t + 1) * P, c * QBLK : (c + 1) * QBLK],
                )

        mm_r = mm.rearrange("(t p) q -> p t q", p=P)
        wv_sb = loadw("wv", wvT)
        for t in range(2):  # chunk b: kt 8..15
            nc.sync.dma_start(
                out=xk_sb[t][:, 1024:2048], in_=xkT[t * P : (t + 1) * P, 1024:2048]
            )
        mk0 = big.tile([P, NKT, QBLK], BF16, tag="mk0", name="mk0")
        for c in range(4):
            nc.sync.dma_start(
                out=mk0[:, 4 * c : 4 * (c + 1), :],
                in_=mm_r[:, 4 * c : 4 * (c + 1), 0:QBLK],
            )
        for t in range(2):  # xq chunk b (only needed by the second q-block)
            nc.sync.dma_start(
                out=xq_sb[t][:, QBLK:NQ], in_=xqT[t * P : (t + 1) * P, QBLK:NQ]
            )
        wo_sb = loadw("wo", woT, dt=BF16)
        mk1 = big.tile([P, NKT, QBLK], BF16, tag="mk1", name="mk1")
        nc.sync.dma_start(out=mk1, in_=mm_r[:, :, QBLK:NQ])
        mks = [mk0, mk1]

        # identity (bf16) for PE transposes + ACT exp-table preload
        ident = const.tile([P, P], BF16, tag="ident", name="ident")
        make_identity(nc, ident)
        warm = wrk.tile([P, 1], F32, tag="warm", name="warm")
        nc.gpsimd.memset(warm, 0.0)
        nc.scalar.activation(warm, warm, EXP)

        qt_sb = [big.tile([P, NQ], BF16, tag=f"qt{t}", name=f"qt{t}") for t in range(2)]
        kt_sb = [big.tile([P, NK], BF16, tag=f"kt{t}", name=f"kt{t}") for t in range(2)]
        v_sb = [None] * NKT

        # ---- compute helpers ----
        def qproj_chunk(t, c):  # q columns c*QBLK:(c+1)*QBLK
            ps = ps_s.tile([P, 2, QBLK], F32, tag="s", name=f"qproj{t}_{c}")
            qs = slice(c * QBLK, (c + 1) * QBLK)
            for ct in range(NCT):
                nc.tensor.matmul(
                    ps[:, 0, :],
                    lhsT=R(wq_sb[ct][:, t * P : (t + 1) * P]),
                    rhs=R(xq_sb[ct][:, qs]),
                    start=(ct == 0),
                    stop=(ct == NCT - 1),
                )
            nc.vector.tensor_copy(qt_sb[t][:, qs], ps[:, 0, :])

        def kproj_chunk(t, c):  # k columns c*1024:(c+1)*1024
            ps = ps_s.tile([P, 2, QBLK], F32, tag="s", name=f"kproj{t}_{c}")
            for nb in range(2):
                off = c * 1024 + nb * QBLK
                for ct in range(NCT):
                    nc.tensor.matmul(
                        ps[:, nb, :],
                        lhsT=R(wk_sb[ct][:, t * P : (t + 1) * P]),
                        rhs=R(xk_sb[ct][:, off : off + QBLK]),
                        start=(ct == 0),
                        stop=(ct == NCT - 1),
                    )
            nc.vector.tensor_copy(
                kt_sb[t][:, c * 1024 : (c + 1) * 1024],
                ps.rearrange("p a b -> p (a b)"),
            )

        def vproj(kt):
            ps = ps_o.tile([P, HID], F32, tag="o", name=f"vproj{kt}")
            for ct in range(NCT):
                nc.tensor.matmul(
                    ps,
                    lhsT=R(xk_sb[ct][:, kt * P : (kt + 1) * P]),
                    rhs=R(wv_sb[ct]),
                    start=(ct == 0),
                    stop=(ct == NCT - 1),
                )
            tl = big.tile([P, NHEAD, VW], BF16, tag=f"v{kt}", name=f"v{kt}")
            nc.gpsimd.tensor_copy(
                tl[:, :, 0:DHEAD], ps.rearrange("p (h d) -> p h d", h=NHEAD)
            )
            nc.gpsimd.memset(tl[:, :, DHEAD : DHEAD + 1], 1.0)
            nc.gpsimd.memset(tl[:, :, DHEAD + 1 : VW], 0.0)
            v_sb[kt] = tl

        # Q-proj chunk a (both t share one psum tile) and the first 1024 K
        # columns of t0, emitted in 512-col pieces so S-matmuls unblock ASAP.
        ps_q = ps_s.tile([P, 2, QBLK], F32, tag="s", name="qproj_a")
        for t in range(2):
            for ct in range(NCT):
                nc.tensor.matmul(
                    ps_q[:, t, :],
                    lhsT=R(wq_sb[ct][:, t * P : (t + 1) * P]),
                    rhs=R(xq_sb[ct][:, 0:QBLK]),
                    start=(ct == 0),
                    stop=(ct == NCT - 1),
                )
            nc.vector.tensor_copy(qt_sb[t][:, 0:QBLK], ps_q[:, t, :])
        ps_k = ps_s.tile([P, 2, QBLK], F32, tag="s", name="kproj_a")
        for c in range(2):
            for ct in range(NCT):
                nc.tensor.matmul(
                    ps_k[:, c, :],
                    lhsT=R(wk_sb[ct][:, 0:P]),
                    rhs=R(xk_sb[ct][:, c * QBLK : (c + 1) * QBLK]),
                    start=(ct == 0),
                    stop=(ct == NCT - 1),
                )
            nc.vector.tensor_copy(kt_sb[0][:, c * QBLK : (c + 1) * QBLK], ps_k[:, c, :])

        # per-slot extra work woven into the first two heads of qb0
        V_SLOT = {g: [2 * g, 2 * g + 1] for g in range(NG)}  # h0 slots
        K_SLOT_H0 = {2: (0, 1)}  # K-proj chunk (t, c) by slot
        K_SLOT_H1 = {0: (1, 0), 1: (1, 1)}

        # ---- attention: one flat slot stream over all (qb, h) pairs ----
        # Each head's drain (last A pair, A^T eviction, transpose, reciprocal,
        # normalize) is deferred into the NEXT head's slot 1 so the ACT engine
        # always has the next head's scores queued.
        anorms_all = {
            qb: [
                wrk.tile([P, HID], BF16, tag=f"an{qt}", name=f"an{qb}_{qt}")
                for qt in range(NQT)
            ]
            for qb in range(NQB)
        }

        def make_S(qb, h, t, po, pts):
            def emit_S(g):
                ps = ps_s.tile([P, 2, QBLK], F32, tag="s", name=f"s{qb}_{h}_{g}")
                for half in range(2):
                    kt = 2 * g + half
                    nc.tensor.matmul(
                        ps[:, half, :],
                        lhsT=kt_sb[t][po : po + DHEAD, kt * P : (kt + 1) * P],
                        rhs=qt_sb[t][po : po + DHEAD, qb * QBLK : (qb + 1) * QBLK],
                        start=True,
                        stop=True,
                    )
                pt = ptp.tile([P, 2, QBLK], BF16, tag="pt", name=f"p{qb}_{h}_{g}")
                nc.scalar.activation(pt, ps, EXP)
                nc.vector.tensor_mul(pt, pt, mks[qb][:, 2 * g : 2 * g + 2, :])
                pts[g] = pt

            return emit_S

        def make_A(h, ps_acc, pts):
            def emit_A(g):
                for half in range(2):
                    kt = 2 * g + half
                    nc.tensor.matmul(
                        ps_acc,
                        lhsT=v_sb[kt][:, h, :],
                        rhs=pts[g][:, half, :],
                        start=(g == 0 and half == 0),
                        stop=(g == NG - 1 and half == 1),
                    )

            return emit_A

        def emit_drain(qb, h, emit_A):
            at = atp.tile([VW, QBLK], BF16, tag="at", name=f"at{qb}_{h}")
            nc.gpsimd.tensor_copy(at, ps_accs[(qb, h)])
            if qb < NQB - 1:
                a_t = wrk.tile([P, NQT, VW], BF16, tag=f"a_t{h}", name=f"a_t{qb}_{h}")
                nc.sync.dma_start_transpose(a_t, at)
            else:
                # tail q-block: PE transposes (lower latency than xbar DMA);
                # lives in the ps_o pool, which is idle between V-proj and the
                # output projections.
                a_t = ps_o.tile([P, NQT, VW], BF16, tag="o", name=f"a_tp{qb}_{h}")
                for qt in range(NQT):
                    nc.tensor.transpose(
                        a_t[:, qt, :], at[:, qt * P : (qt + 1) * P], ident[0:VW, 0:VW]
                    )
            rec = wrk.tile([P, NQT, 1], F32, tag=f"rec{h}", name=f"rec{qb}_{h}")
            nc.vector.reciprocal(rec, a_t[:, :, DHEAD : DHEAD + 1])
            # normalize immediately: frees a_t (PSUM in the tail case)
            for qt in range(NQT):
                nc.vector.tensor_scalar_mul(
                    anorms_all[qb][qt][:, h * DHEAD : (h + 1) * DHEAD],
                    a_t[:, qt, 0:DHEAD],
                    rec[:, qt, :],
                )

        def emit_qtloop(qb):
            for qt in range(NQT):
                anorm = anorms_all[qb][qt]
                if qb < NQB - 1:
                    att = wrk.tile([P, NCT, P], BF16, tag="att", name=f"att{qb}_{qt}")
                    nc.sync.dma_start_transpose(att, anorm)
                else:
                    attp = ps_o.tile([P, NCT, P], BF16, tag="o", name=f"attp{qb}_{qt}")
                    for ct in range(NCT):
                        nc.tensor.transpose(
                            attp[:, ct, :], anorm[:, ct * P : (ct + 1) * P], ident
                        )
                    att = wrk.tile([P, NCT, P], BF16, tag="att", name=f"att{qb}_{qt}")
                    nc.vector.tensor_copy(att, attp)
                ps_out = ps_o.tile([P, HID], F32, tag="o", name=f"o{qb}_{qt}")
                for ct in range(NCT):
                    nc.tensor.matmul(
                        ps_out,
                        lhsT=att[:, ct, :],
                        rhs=wo_sb[ct],
                        start=(ct == 0),
                        stop=(ct == NCT - 1),
                    )
                ob = obp.tile([P, HID], F32, tag="ob", name=f"ob{qb}_{qt}")
                nc.gpsimd.tensor_copy(ob, ps_out)
                q0 = qb * QBLK + qt * P
                nc.sync.dma_start(out=out_d[q0 : q0 + P, :], in_=ob)

        # A-matmuls trail their S-group by an adaptive slot count (deep early,
        # while the mask DMAs are still streaming in; shallow later so the
        # final drain is short).  Each popped A that closes a head's
        # accumulation immediately triggers that head's drain.
        ps_accs = {}
        pending = []  # FIFO of (qb, h, g, emit_A, is_last_of_head)

        def pop_A():
            qb_, h_, g_, eA, last = pending.pop(0)
            eA(g_)
            if last:
                emit_drain(qb_, h_, eA)
                if h_ == NHEAD - 1:
                    emit_qtloop(qb_)

        for qb in range(NQB):
            for h in range(NHEAD):
                t, po = h // 2, (h % 2) * DHEAD
                ps_accs[(qb, h)] = ps_a.tile([VW, QBLK], F32, tag="a", name=f"a{qb}_{h}")
                pts = [None] * NG
                emit_S = make_S(qb, h, t, po, pts)
                emit_A = make_A(h, ps_accs[(qb, h)], pts)
                depth = 4 if (qb == 0 and h < 2) else 2
                for g in range(NG):
                    emit_S(g)
                    if qb == 0 and h == 0:
                        for kt in V_SLOT[g]:
                            vproj(kt)
                        if g in K_SLOT_H0:
                            kproj_chunk(*K_SLOT_H0[g])
                    if qb == 0 and h == 1 and g in K_SLOT_H1:
                        kproj_chunk(*K_SLOT_H1[g])
                    if qb == 0 and h == 3 and g == 3:
                        qproj_chunk(0, 1)
                    if qb == 0 and h == 3 and g == 5:
                        qproj_chunk(1, 1)
                    pending.append((qb, h, g, emit_A, g == NG - 1))
                    while len(pending) > depth:
                        pop_A()
        while pending:
            pop_A()
    nc.compile()
    return nc


_NC_CACHE = {}
_last_in_maps = None


def _get_nc() -> bass.Bass:
    if "nc" not in _NC_CACHE:
        _NC_CACHE["nc"] = build()
    return _NC_CACHE["nc"]


def kernel(q_hidden_states, k_hidden_states, attention_mask, align_mask, Wq, Wk, Wv, Wo):
    from concourse.bass_utils import run_bass_kernel_spmd

    q_hidden_states = np.asarray(q_hidden_states, np.float32)
    k_hidden_states = np.asarray(k_hidden_states, np.float32)
    attention_mask = np.asarray(attention_mask, np.float32)
    align_mask = np.asarray(align_mask)
    B, Q, _ = q_hidden_states.shape
    qh_len = Q // 2  # 1024

    nc = _get_nc()

    wq = np.ascontiguousarray(np.asarray(Wq, np.float32).T) / np.float32(8.0)
    wk = np.ascontiguousarray(np.asarray(Wk, np.float32).T)
    wv = np.ascontiguousarray(np.asarray(Wv, np.float32).T)
    wo = np.ascontiguousarray(np.asarray(Wo, np.float32).T).astype(ml_dtypes.bfloat16)

    use_mask = bool(np.any(attention_mask))

    in_maps = []
    for core in range(8):
        b, qh = divmod(core, 2)
        qsl = slice(qh * qh_len, (qh + 1) * qh_len)
        # multiplicative mask: align * exp(attention_mask)  (exact: the
        # reference adds attention_mask pre-exp and zeroes where align==0)
        mmask = align_mask[b, :, qsl].astype(np.float32)
        if use_mask:
            mmask = mmask * np.exp(
                np.ascontiguousarray(attention_mask[b, 0, qsl, :].T), dtype=np.float32
            )
        m = {
            "xqT": np.ascontiguousarray(q_hidden_states[b, qsl].T),
            "xkT": np.ascontiguousarray(k_hidden_states[b].T),
            "mm": np.ascontiguousarray(mmask.astype(ml_dtypes.bfloat16)),
            "wqT": wq,
            "wkT": wk,
            "wvT": wv,
            "woT": wo,
        }
        in_maps.append(m)

    global _last_in_maps
    _last_in_maps = in_maps
    res = run_bass_kernel_spmd(nc, in_maps, list(range(8))).results
    out = np.empty((B, Q, HID), np.float32)
    for core in range(8):
        b, qh = divmod(core, 2)
        out[b, qh * qh_len : (qh + 1) * qh_len] = res[core]["out"]
    return out


# revision 15
# speedup vs baseline: 1.0379x; 1.0038x over previous
"""KgAdapterCrossAttention kernel for 8 trn2 NeuronCores.

Sharding: core = (batch b, query-half qh).  Each core computes attention for
1024 queries of one batch element against all 2048 keys.

Engine plan (per core):
  - PE: projections in float32r (1 cyc/row at ap>=256), attention matmuls in
    bf16.  Scores computed transposed S^T [k, q] per head; A computed as
    A^T = V''^T P^T with ap_size=512 and a ones-column in V'' providing the
    softmax denominator (padded to 80 cols so transposes are xbar-friendly).
  - ACT: exp over 2-bank PSUM tiles [128, 2, 512] writing bf16 P^T to SBUF.
    ACT is the bottleneck engine (~66us of exp); everything else is
    scheduled to keep it saturated from ~6us to the end.
  - DVE: multiplicative align-mask (bf16, 2x mode), projection evictions,
    per-q normalize.
  - GPSIMD: PSUM evictions (V'', A^T, output tiles).
  - DMA xbar: transposes (A^T -> A, anorm -> att) for the first q-block;
    the last q-block uses PE transposes to shorten the drain tail.
  - Startup: need-ordered chunked DMAs; V-proj and the K-proj tail are
    interleaved into the first head's pipeline slots.
  - softmax without max-subtraction (scores ~N(0,1)); attention_mask folded
    into the multiplicative mask on host: align * exp(attn_mask) (exact).
"""

import os
import sys

import numpy as np

try:
    import concourse.bass as bass
except ImportError:
    for _p in ("/opt/trn_rl_repo", os.path.expanduser("~/.axon_site/_ro/trn_rl_repo")):
        if os.path.isdir(_p) and _p not in sys.path:
            sys.path.insert(0, _p)
    import concourse.bass as bass

import ml_dtypes
import concourse.mybir as mybir
import concourse.tile as tile
from concourse import bacc
from concourse.masks import make_identity
from contextlib import ExitStack

F32 = mybir.dt.float32
F32R = mybir.dt.float32r
BF16 = mybir.dt.bfloat16
EXP = mybir.ActivationFunctionType.Exp

P = 128
HID = 256
NHEAD = 4
DHEAD = 64
NQ = 1024  # queries per core
NK = 2048  # keys (full)
QBLK = 512
NQB = NQ // QBLK  # 2
NKT = NK // P  # 16
NG = NKT // 2  # 8 kt-pair groups per (qb, h)
NCT = HID // P  # 2 contraction tiles over hidden
VW = 80  # A^T width: 64 V cols + 1 ones col + 15 zero pad (xbar wants %16)
NQT = QBLK // P  # 4


def build() -> bass.Bass:
    nc = bacc.Bacc()
    xqT = nc.declare_dram_parameter("xqT", [HID, NQ], F32, isOutput=False)
    xkT = nc.declare_dram_parameter("xkT", [HID, NK], F32, isOutput=False)
    mm = nc.declare_dram_parameter("mm", [NK, NQ], BF16, isOutput=False)
    wqT = nc.declare_dram_parameter("wqT", [HID, HID], F32, isOutput=False)
    wkT = nc.declare_dram_parameter("wkT", [HID, HID], F32, isOutput=False)
    wvT = nc.declare_dram_parameter("wvT", [HID, HID], F32, isOutput=False)
    woT = nc.declare_dram_parameter("woT", [HID, HID], BF16, isOutput=False)
    out_d = nc.declare_dram_parameter("out", [NQ, HID], F32, isOutput=True)

    def R(ap):
        return ap.bitcast(F32R)

    with tile.TileContext(nc) as tc, ExitStack() as ctx:
        const = ctx.enter_context(tc.tile_pool(name="const", bufs=1))
        big = ctx.enter_context(tc.tile_pool(name="big", bufs=1))
        ptp = ctx.enter_context(tc.tile_pool(name="ptp", bufs=14))
        atp = ctx.enter_context(tc.tile_pool(name="atp", bufs=2))
        wrk = ctx.enter_context(tc.tile_pool(name="wrk", bufs=2))
        obp = ctx.enter_context(tc.tile_pool(name="obp", bufs=2))
        ps_s = ctx.enter_context(tc.tile_pool(name="ps_s", bufs=2, space="PSUM"))
        ps_a = ctx.enter_context(tc.tile_pool(name="ps_a", bufs=2, space="PSUM"))
        ps_o = ctx.enter_context(tc.tile_pool(name="ps_o", bufs=2, space="PSUM"))

        # ---- DMA emission, ordered by first need ----
        def loadw(name, src, dt=F32, cols=None):
            ts = []
            for t in range(2):
                tl = const.tile([P, HID], dt, tag=f"{name}{t}", name=f"{name}{t}")
                nc.sync.dma_start(out=tl, in_=src[t * P : (t + 1) * P, :])
                ts.append(tl)
            return ts

        wq_sb = loadw("wq", wqT)
        wk_sb = loadw("wk", wkT)
        xk_sb = [big.tile([P, NK], F32, tag=f"xk{t}", name=f"xk{t}") for t in range(2)]
        for t in range(2):  # kt 0..3 — K-proj has the longest startup chain
            nc.sync.dma_start(out=xk_sb[t][:, 0:QBLK], in_=xkT[t * P : (t + 1) * P, 0:QBLK])
        xq_sb = [big.tile([P, NQ], F32, tag=f"xq{t}", name=f"xq{t}") for t in range(2)]
        for t in range(2):  # first q-block's columns
            nc.sync.dma_start(out=xq_sb[t][:, 0:QBLK], in_=xqT[t * P : (t + 1) * P, 0:QBLK])
        for t in range(2):  # kt 4..7
            nc.sync.dma_start(
                out=xk_sb[t][:, QBLK : 2 * QBLK],
                in_=xkT[t * P : (t + 1) * P, QBLK : 2 * QBLK],
            )

        mm_r = mm.rearrange("(t p) q -> p t q", p=P)
        wv_sb = loadw("wv", wvT)
        for t in range(2):  # chunk b: kt 8..15
            nc.sync.dma_start(
                out=xk_sb[t][:, 1024:2048], in_=xkT[t * P : (t + 1) * P, 1024:2048]
            )
        mk0 = big.tile([P, NKT, QBLK], BF16, tag="mk0", name="mk0")
        for c in range(4):
            nc.sync.dma_start(
                out=mk0[:, 4 * c : 4 * (c + 1), :],
                in_=mm_r[:, 4 * c : 4 * (c + 1), 0:QBLK],
            )
        for t in range(2):  # xq chunk b (only needed by the second q-block)
            nc.sync.dma_start(
                out=xq_sb[t][:, QBLK:NQ], in_=xqT[t * P : (t + 1) * P, QBLK:NQ]
            )
        wo_sb = loadw("wo", woT, dt=BF16)
        mk1 = big.tile([P, NKT, QBLK], BF16, tag="mk1", name="mk1")
        nc.sync.dma_start(out=mk1, in_=mm_r[:, :, QBLK:NQ])
        mks = [mk0, mk1]

        # identity (bf16) for PE transposes + ACT exp-table preload
        ident = const.tile([P, P], BF16, tag="ident", name="ident")
        make_identity(nc, ident)
        warm = wrk.tile([P, 1], F32, tag="warm", name="warm")
        nc.gpsimd.memset(warm, 0.0)
        nc.scalar.activation(warm, warm, EXP)

        qt_sb = [big.tile([P, NQ], BF16, tag=f"qt{t}", name=f"qt{t}") for t in range(2)]
        kt_sb = [big.tile([P, NK], BF16, tag=f"kt{t}", name=f"kt{t}") for t in range(2)]
        v_sb = [None] * NKT

        # ---- compute helpers ----
        def qproj_chunk(t, c):  # q columns c*QBLK:(c+1)*QBLK
            ps = ps_s.tile([P, 2, QBLK], F32, tag="s", name=f"qproj{t}_{c}")
            qs = slice(c * QBLK, (c + 1) * QBLK)
            for ct in range(NCT):
                nc.tensor.matmul(
                    ps[:, 0, :],
                    lhsT=R(wq_sb[ct][:, t * P : (t + 1) * P]),
                    rhs=R(xq_sb[ct][:, qs]),
                    start=(ct == 0),
                    stop=(ct == NCT - 1),
                )
            nc.vector.tensor_copy(qt_sb[t][:, qs], ps[:, 0, :])

        def kproj_chunk(t, c):  # k columns c*1024:(c+1)*1024
            ps = ps_s.tile([P, 2, QBLK], F32, tag="s", name=f"kproj{t}_{c}")
            for nb in range(2):
                off = c * 1024 + nb * QBLK
                for ct in range(NCT):
                    nc.tensor.matmul(
                        ps[:, nb, :],
                        lhsT=R(wk_sb[ct][:, t * P : (t + 1) * P]),
                        rhs=R(xk_sb[ct][:, off : off + QBLK]),
                        start=(ct == 0),
                        stop=(ct == NCT - 1),
                    )
            nc.vector.tensor_copy(
                kt_sb[t][:, c * 1024 : (c + 1) * 1024],
                ps.rearrange("p a b -> p (a b)"),
            )

        def vproj(kt):
            ps = ps_o.tile([P, HID], F32, tag="o", name=f"vproj{kt}")
            for ct in range(NCT):
                nc.tensor.matmul(
                    ps,
                    lhsT=R(xk_sb[ct][:, kt * P : (kt + 1) * P]),
                    rhs=R(wv_sb[ct]),
                    start=(ct == 0),
                    stop=(ct == NCT - 1),
                )
            tl = big.tile([P, NHEAD, VW], BF16, tag=f"v{kt}", name=f"v{kt}")
            nc.gpsimd.tensor_copy(
                tl[:, :, 0:DHEAD], ps.rearrange("p (h d) -> p h d", h=NHEAD)
            )
            nc.gpsimd.memset(tl[:, :, DHEAD : DHEAD + 1], 1.0)
            nc.gpsimd.memset(tl[:, :, DHEAD + 1 : VW], 0.0)
            v_sb[kt] = tl

        # Startup: K cols 0:512 of t0 first (longest chain to the first
        # S-matmul), then Q chunk a, then K cols 512:1024 — each 512-col piece
        # in its own psum tile so the first S-group only waits on what it reads.
        def k512(c):
            ps = ps_s.tile([P, 2, QBLK], F32, tag="s", name=f"kproj_a{c}")
            for ct in range(NCT):
                nc.tensor.matmul(
                    ps[:, 0, :],
                    lhsT=R(wk_sb[ct][:, 0:P]),
                    rhs=R(xk_sb[ct][:, c * QBLK : (c + 1) * QBLK]),
                    start=(ct == 0),
                    stop=(ct == NCT - 1),
                )
            nc.vector.tensor_copy(kt_sb[0][:, c * QBLK : (c + 1) * QBLK], ps[:, 0, :])

        k512(0)
        ps_q = ps_s.tile([P, 2, QBLK], F32, tag="s", name="qproj_a")
        for t in range(2):
            for ct in range(NCT):
                nc.tensor.matmul(
                    ps_q[:, t, :],
                    lhsT=R(wq_sb[ct][:, t * P : (t + 1) * P]),
                    rhs=R(xq_sb[ct][:, 0:QBLK]),
                    start=(ct == 0),
                    stop=(ct == NCT - 1),
                )
            nc.vector.tensor_copy(qt_sb[t][:, 0:QBLK], ps_q[:, t, :])
        k512(1)

        # per-slot extra work woven into the first two heads of qb0
        V_SLOT = {g: [2 * g, 2 * g + 1] for g in range(NG)}  # h0 slots
        K_SLOT_H0 = {2: (0, 1)}  # K-proj chunk (t, c) by slot
        K_SLOT_H1 = {0: (1, 0), 1: (1, 1)}

        # ---- attention: one flat slot stream over all (qb, h) pairs ----
        # Each head's drain (last A pair, A^T eviction, transpose, reciprocal,
        # normalize) is deferred into the NEXT head's slot 1 so the ACT engine
        # always has the next head's scores queued.
        anorms_all = {
            qb: [
                wrk.tile([P, HID], BF16, tag=f"an{qt}", name=f"an{qb}_{qt}")
                for qt in range(NQT)
            ]
            for qb in range(NQB)
        }

        def make_S(qb, h, t, po, pts):
            def emit_S(g):
                ps = ps_s.tile([P, 2, QBLK], F32, tag="s", name=f"s{qb}_{h}_{g}")
                for half in range(2):
                    kt = 2 * g + half
                    nc.tensor.matmul(
                        ps[:, half, :],
                        lhsT=kt_sb[t][po : po + DHEAD, kt * P : (kt + 1) * P],
                        rhs=qt_sb[t][po : po + DHEAD, qb * QBLK : (qb + 1) * QBLK],
                        start=True,
                        stop=True,
                    )
                pt = ptp.tile([P, 2, QBLK], BF16, tag="pt", name=f"p{qb}_{h}_{g}")
                nc.scalar.activation(pt, ps, EXP)
                nc.vector.tensor_mul(pt, pt, mks[qb][:, 2 * g : 2 * g + 2, :])
                pts[g] = pt

            return emit_S

        def make_A(h, ps_acc, pts):
            def emit_A(g):
                for half in range(2):
                    kt = 2 * g + half
                    nc.tensor.matmul(
                        ps_acc,
                        lhsT=v_sb[kt][:, h, :],
                        rhs=pts[g][:, half, :],
                        start=(g == 0 and half == 0),
                        stop=(g == NG - 1 and half == 1),
                    )

            return emit_A

        def emit_drain(qb, h, emit_A):
            at = atp.tile([VW, QBLK], BF16, tag="at", name=f"at{qb}_{h}")
            nc.gpsimd.tensor_copy(at, ps_accs[(qb, h)])
            if qb < NQB - 1:
                a_t = wrk.tile([P, NQT, VW], BF16, tag=f"a_t{h}", name=f"a_t{qb}_{h}")
                nc.sync.dma_start_transpose(a_t, at)
            else:
                # tail q-block: PE transposes (lower latency than xbar DMA);
                # lives in the ps_o pool, which is idle between V-proj and the
                # output projections.
                a_t = ps_o.tile([P, NQT, VW], BF16, tag="o", name=f"a_tp{qb}_{h}")
                for qt in range(NQT):
                    nc.tensor.transpose(
                        a_t[:, qt, :], at[:, qt * P : (qt + 1) * P], ident[0:VW, 0:VW]
                    )
            rec = wrk.tile([P, NQT, 1], F32, tag=f"rec{h}", name=f"rec{qb}_{h}")
            nc.vector.reciprocal(rec, a_t[:, :, DHEAD : DHEAD + 1])
            # normalize immediately: frees a_t (PSUM in the tail case)
            for qt in range(NQT):
                nc.vector.tensor_scalar_mul(
                    anorms_all[qb][qt][:, h * DHEAD : (h + 1) * DHEAD],
                    a_t[:, qt, 0:DHEAD],
                    rec[:, qt, :],
                )

        def emit_qtloop(qb):
            for qt in range(NQT):
                anorm = anorms_all[qb][qt]
                if qb < NQB - 1:
                    att = wrk.tile([P, NCT, P], BF16, tag="att", name=f"att{qb}_{qt}")
                    nc.sync.dma_start_transpose(att, anorm)
                else:
                    attp = ps_o.tile([P, NCT, P], BF16, tag="o", name=f"attp{qb}_{qt}")
                    for ct in range(NCT):
                        nc.tensor.transpose(
                            attp[:, ct, :], anorm[:, ct * P : (ct + 1) * P], ident
                        )
                    att = wrk.tile([P, NCT, P], BF16, tag="att", name=f"att{qb}_{qt}")
                    nc.vector.tensor_copy(att, attp)
                ps_out = ps_o.tile([P, HID], F32, tag="o", name=f"o{qb}_{qt}")
                for ct in range(NCT):
                    nc.tensor.matmul(
                        ps_out,
                        lhsT=att[:, ct, :],
                        rhs=wo_sb[ct],
                        start=(ct == 0),
                        stop=(ct == NCT - 1),
                    )
                ob = obp.tile([P, HID], F32, tag="ob", name=f"ob{qb}_{qt}")
                nc.gpsimd.tensor_copy(ob, ps_out)
                q0 = qb * QBLK + qt * P
                nc.sync.dma_start(out=out_d[q0 : q0 + P, :], in_=ob)

        # A-matmuls trail their S-group by an adaptive slot count (deep early,
        # while the mask DMAs are still streaming in; shallow later so the
        # final drain is short).  Each popped A that closes a head's
        # accumulation immediately triggers that head's drain.
        ps_accs = {}
        pending = []  # FIFO of (qb, h, g, emit_A, is_last_of_head)

        def pop_A():
            qb_, h_, g_, eA, last = pending.pop(0)
            eA(g_)
            if last:
                emit_drain(qb_, h_, eA)
                if h_ == NHEAD - 1:
                    emit_qtloop(qb_)

        for qb in range(NQB):
            for h in range(NHEAD):
                t, po = h // 2, (h % 2) * DHEAD
                ps_accs[(qb, h)] = ps_a.tile([VW, QBLK], F32, tag="a", name=f"a{qb}_{h}")
                pts = [None] * NG
                emit_S = make_S(qb, h, t, po, pts)
                emit_A = make_A(h, ps_accs[(qb, h)], pts)
                depth = 4 if (qb == 0 and h < 2) else 2
                for g in range(NG):
                    emit_S(g)
                    if qb == 0 and h == 0:
                        for kt in V_SLOT[g]:
                            vproj(kt)
                        if g in K_SLOT_H0:
                            kproj_chunk(*K_SLOT_H0[g])
                    if qb == 0 and h == 1 and g in K_SLOT_H1:
                        kproj_chunk(*K_SLOT_H1[g])
                    if qb == 0 and h == 3 and g == 3:
                        qproj_chunk(0, 1)
                    if qb == 0 and h == 3 and g == 5:
                        qproj_chunk(1, 1)
                    pending.append((qb, h, g, emit_A, g == NG - 1))
                    while len(pending) > depth:
                        pop_A()
        while pending:
            pop_A()
    nc.compile()
    return nc


_NC_CACHE = {}
_last_in_maps = None


def _get_nc() -> bass.Bass:
    if "nc" not in _NC_CACHE:
        _NC_CACHE["nc"] = build()
    return _NC_CACHE["nc"]


def kernel(q_hidden_states, k_hidden_states, attention_mask, align_mask, Wq, Wk, Wv, Wo):
    from concourse.bass_utils import run_bass_kernel_spmd

    q_hidden_states = np.asarray(q_hidden_states, np.float32)
    k_hidden_states = np.asarray(k_hidden_states, np.float32)
    attention_mask = np.asarray(attention_mask, np.float32)
    align_mask = np.asarray(align_mask)
    B, Q, _ = q_hidden_states.shape
    qh_len = Q // 2  # 1024

    nc = _get_nc()

    wq = np.ascontiguousarray(np.asarray(Wq, np.float32).T) / np.float32(8.0)
    wk = np.ascontiguousarray(np.asarray(Wk, np.float32).T)
    wv = np.ascontiguousarray(np.asarray(Wv, np.float32).T)
    wo = np.ascontiguousarray(np.asarray(Wo, np.float32).T).astype(ml_dtypes.bfloat16)

    use_mask = bool(np.any(attention_mask))

    in_maps = []
    for core in range(8):
        b, qh = divmod(core, 2)
        qsl = slice(qh * qh_len, (qh + 1) * qh_len)
        # multiplicative mask: align * exp(attention_mask)  (exact: the
        # reference adds attention_mask pre-exp and zeroes where align==0)
        mmask = align_mask[b, :, qsl].astype(np.float32)
        if use_mask:
            mmask = mmask * np.exp(
                np.ascontiguousarray(attention_mask[b, 0, qsl, :].T), dtype=np.float32
            )
        m = {
            "xqT": np.ascontiguousarray(q_hidden_states[b, qsl].T),
            "xkT": np.ascontiguousarray(k_hidden_states[b].T),
            "mm": np.ascontiguousarray(mmask.astype(ml_dtypes.bfloat16)),
            "wqT": wq,
            "wkT": wk,
            "wvT": wv,
            "woT": wo,
        }
        in_maps.append(m)

    global _last_in_maps
    _last_in_maps = in_maps
    res = run_bass_kernel_spmd(nc, in_maps, list(range(8))).results
    out = np.empty((B, Q, HID), np.float32)
    for core in range(8):
        b, qh = divmod(core, 2)
        out[b, qh * qh_len : (qh + 1) * qh_len] = res[core]["out"]
    return out


# revision 18
# speedup vs baseline: 1.0395x; 1.0015x over previous
"""KgAdapterCrossAttention kernel for 8 trn2 NeuronCores.

Sharding: core = (batch b, query-half qh).  Each core computes attention for
1024 queries of one batch element against all 2048 keys.

Engine plan (per core):
  - PE: projections in float32r (1 cyc/row at ap>=256), attention matmuls in
    bf16.  Scores computed transposed S^T [k, q] per head; A computed as
    A^T = V''^T P^T with ap_size=512 and a ones-column in V'' providing the
    softmax denominator (padded to 80 cols so transposes are xbar-friendly).
  - ACT: exp over 2-bank PSUM tiles [128, 2, 512] writing bf16 P^T to SBUF.
    ACT is the bottleneck engine (~66us of exp); everything else is
    scheduled to keep it saturated from ~6us to the end.
  - DVE: multiplicative align-mask (bf16, 2x mode), projection evictions,
    per-q normalize.
  - GPSIMD: PSUM evictions (V'', A^T, output tiles).
  - DMA xbar: transposes (A^T -> A, anorm -> att) for the first q-block;
    the last q-block uses PE transposes to shorten the drain tail.
  - Startup: need-ordered chunked DMAs; V-proj and the K-proj tail are
    interleaved into the first head's pipeline slots.
  - softmax without max-subtraction (scores ~N(0,1)); attention_mask folded
    into the multiplicative mask on host: align * exp(attn_mask) (exact).
"""

import os
import sys

import numpy as np

try:
    import concourse.bass as bass
except ImportError:
    for _p in ("/opt/trn_rl_repo", os.path.expanduser("~/.axon_site/_ro/trn_rl_repo")):
        if os.path.isdir(_p) and _p not in sys.path:
            sys.path.insert(0, _p)
    import concourse.bass as bass

import ml_dtypes
import concourse.mybir as mybir
import concourse.tile as tile
from concourse import bacc
from concourse.masks import make_identity
from contextlib import ExitStack

F32 = mybir.dt.float32
F32R = mybir.dt.float32r
BF16 = mybir.dt.bfloat16
EXP = mybir.ActivationFunctionType.Exp

P = 128
HID = 256
NHEAD = 4
DHEAD = 64
NQ = 1024  # queries per core
NK = 2048  # keys (full)
QBLK = 512
NQB = NQ // QBLK  # 2
NKT = NK // P  # 16
NG = NKT // 2  # 8 kt-pair groups per (qb, h)
NCT = HID // P  # 2 contraction tiles over hidden
VW = 80  # A^T width: 64 V cols + 1 ones col + 15 zero pad (xbar wants %16)
NQT = QBLK // P  # 4


def build() -> bass.Bass:
    nc = bacc.Bacc()
    xqT = nc.declare_dram_parameter("xqT", [HID, NQ], BF16, isOutput=False)
    xkT = nc.declare_dram_parameter("xkT", [HID, NK], BF16, isOutput=False)
    mm = nc.declare_dram_parameter("mm", [NK, NQ], BF16, isOutput=False)
    wqT = nc.declare_dram_parameter("wqT", [HID, HID], BF16, isOutput=False)
    wkT = nc.declare_dram_parameter("wkT", [HID, HID], BF16, isOutput=False)
    wvT = nc.declare_dram_parameter("wvT", [HID, HID], BF16, isOutput=False)
    woT = nc.declare_dram_parameter("woT", [HID, HID], BF16, isOutput=False)
    out_d = nc.declare_dram_parameter("out", [NQ, HID], F32, isOutput=True)

    def R(ap):
        return ap.bitcast(F32R)

    with tile.TileContext(nc) as tc, ExitStack() as ctx:
        const = ctx.enter_context(tc.tile_pool(name="const", bufs=1))
        big = ctx.enter_context(tc.tile_pool(name="big", bufs=1))
        ptp = ctx.enter_context(tc.tile_pool(name="ptp", bufs=14))
        atp = ctx.enter_context(tc.tile_pool(name="atp", bufs=2))
        wrk = ctx.enter_context(tc.tile_pool(name="wrk", bufs=2))
        obp = ctx.enter_context(tc.tile_pool(name="obp", bufs=2))
        ps_s = ctx.enter_context(tc.tile_pool(name="ps_s", bufs=2, space="PSUM"))
        ps_a = ctx.enter_context(tc.tile_pool(name="ps_a", bufs=2, space="PSUM"))
        ps_o = ctx.enter_context(tc.tile_pool(name="ps_o", bufs=2, space="PSUM"))

        # ---- DMA emission, ordered by first need ----
        def loadw(name, src, dt=BF16, cols=None):
            ts = []
            for t in range(2):
                tl = const.tile([P, HID], dt, tag=f"{name}{t}", name=f"{name}{t}")
                nc.sync.dma_start(out=tl, in_=src[t * P : (t + 1) * P, :])
                ts.append(tl)
            return ts

        wq_sb = loadw("wq", wqT)
        wk_sb = loadw("wk", wkT)
        xk_sb = [big.tile([P, NK], BF16, tag=f"xk{t}", name=f"xk{t}") for t in range(2)]
        for t in range(2):  # kt 0..3 — K-proj has the longest startup chain
            nc.sync.dma_start(out=xk_sb[t][:, 0:QBLK], in_=xkT[t * P : (t + 1) * P, 0:QBLK])
        xq_sb = [big.tile([P, NQ], BF16, tag=f"xq{t}", name=f"xq{t}") for t in range(2)]
        for t in range(2):  # first q-block's columns
            nc.sync.dma_start(out=xq_sb[t][:, 0:QBLK], in_=xqT[t * P : (t + 1) * P, 0:QBLK])
        for t in range(2):  # kt 4..7
            nc.sync.dma_start(
                out=xk_sb[t][:, QBLK : 2 * QBLK],
                in_=xkT[t * P : (t + 1) * P, QBLK : 2 * QBLK],
            )

        mm_r = mm.rearrange("(t p) q -> p t q", p=P)
        wv_sb = loadw("wv", wvT)
        for t in range(2):  # chunk b: kt 8..15
            nc.sync.dma_start(
                out=xk_sb[t][:, 1024:2048], in_=xkT[t * P : (t + 1) * P, 1024:2048]
            )
        mk0 = big.tile([P, NKT, QBLK], BF16, tag="mk0", name="mk0")
        for c in range(4):
            nc.sync.dma_start(
                out=mk0[:, 4 * c : 4 * (c + 1), :],
                in_=mm_r[:, 4 * c : 4 * (c + 1), 0:QBLK],
            )
        for t in range(2):  # xq chunk b (only needed by the second q-block)
            nc.sync.dma_start(
                out=xq_sb[t][:, QBLK:NQ], in_=xqT[t * P : (t + 1) * P, QBLK:NQ]
            )
        wo_sb = loadw("wo", woT)
        mk1 = big.tile([P, NKT, QBLK], BF16, tag="mk1", name="mk1")
        nc.sync.dma_start(out=mk1, in_=mm_r[:, :, QBLK:NQ])
        mks = [mk0, mk1]

        # identity (bf16) for PE transposes + ACT exp-table preload
        ident = const.tile([P, P], BF16, tag="ident", name="ident")
        make_identity(nc, ident)
        warm = wrk.tile([P, 1], F32, tag="warm", name="warm")
        nc.gpsimd.memset(warm, 0.0)
        nc.scalar.activation(warm, warm, EXP)

        qt_sb = [big.tile([P, NQ], BF16, tag=f"qt{t}", name=f"qt{t}") for t in range(2)]
        kt_sb = [big.tile([P, NK], BF16, tag=f"kt{t}", name=f"kt{t}") for t in range(2)]
        v_sb = [None] * NKT

        # ---- compute helpers ----
        def qproj_chunk(t, c):  # q columns c*QBLK:(c+1)*QBLK
            ps = ps_s.tile([P, 2, QBLK], F32, tag="s", name=f"qproj{t}_{c}")
            qs = slice(c * QBLK, (c + 1) * QBLK)
            for ct in range(NCT):
                nc.tensor.matmul(
                    ps[:, 0, :],
                    lhsT=wq_sb[ct][:, t * P : (t + 1) * P],
                    rhs=xq_sb[ct][:, qs],
                    start=(ct == 0),
                    stop=(ct == NCT - 1),
                )
            nc.vector.tensor_copy(qt_sb[t][:, qs], ps[:, 0, :])

        def kproj_chunk(t, c):  # k columns c*1024:(c+1)*1024
            ps = ps_s.tile([P, 2, QBLK], F32, tag="s", name=f"kproj{t}_{c}")
            for nb in range(2):
                off = c * 1024 + nb * QBLK
                for ct in range(NCT):
                    nc.tensor.matmul(
                        ps[:, nb, :],
                        lhsT=wk_sb[ct][:, t * P : (t + 1) * P],
                        rhs=xk_sb[ct][:, off : off + QBLK],
                        start=(ct == 0),
                        stop=(ct == NCT - 1),
                    )
            nc.vector.tensor_copy(
                kt_sb[t][:, c * 1024 : (c + 1) * 1024],
                ps.rearrange("p a b -> p (a b)"),
            )

        def vproj(kt):
            ps = ps_o.tile([P, HID], F32, tag="o", name=f"vproj{kt}")
            for ct in range(NCT):
                nc.tensor.matmul(
                    ps,
                    lhsT=xk_sb[ct][:, kt * P : (kt + 1) * P],
                    rhs=wv_sb[ct],
                    start=(ct == 0),
                    stop=(ct == NCT - 1),
                )
            tl = big.tile([P, NHEAD, VW], BF16, tag=f"v{kt}", name=f"v{kt}")
            nc.gpsimd.tensor_copy(
                tl[:, :, 0:DHEAD], ps.rearrange("p (h d) -> p h d", h=NHEAD)
            )
            nc.gpsimd.memset(tl[:, :, DHEAD : DHEAD + 1], 1.0)
            nc.gpsimd.memset(tl[:, :, DHEAD + 1 : VW], 0.0)
            v_sb[kt] = tl

        # Startup: K cols 0:512 of t0 first (longest chain to the first
        # S-matmul), then Q chunk a, then K cols 512:1024 — each 512-col piece
        # in its own psum tile so the first S-group only waits on what it reads.
        def k512(c):
            ps = ps_s.tile([P, 2, QBLK], F32, tag="s", name=f"kproj_a{c}")
            for ct in range(NCT):
                nc.tensor.matmul(
                    ps[:, 0, :],
                    lhsT=wk_sb[ct][:, 0:P],
                    rhs=xk_sb[ct][:, c * QBLK : (c + 1) * QBLK],
                    start=(ct == 0),
                    stop=(ct == NCT - 1),
                )
            nc.vector.tensor_copy(kt_sb[0][:, c * QBLK : (c + 1) * QBLK], ps[:, 0, :])

        k512(0)
        ps_q = ps_s.tile([P, 2, QBLK], F32, tag="s", name="qproj_a")
        for t in range(2):
            for ct in range(NCT):
                nc.tensor.matmul(
                    ps_q[:, t, :],
                    lhsT=wq_sb[ct][:, t * P : (t + 1) * P],
                    rhs=xq_sb[ct][:, 0:QBLK],
                    start=(ct == 0),
                    stop=(ct == NCT - 1),
                )
            nc.vector.tensor_copy(qt_sb[t][:, 0:QBLK], ps_q[:, t, :])
        k512(1)

        # per-slot extra work woven into the first two heads of qb0
        V_SLOT = {g: [2 * g, 2 * g + 1] for g in range(NG)}  # h0 slots
        K_SLOT_H0 = {2: (0, 1)}  # K-proj chunk (t, c) by slot
        K_SLOT_H1 = {0: (1, 0), 1: (1, 1)}

        # ---- attention: one flat slot stream over all (qb, h) pairs ----
        # Each head's drain (last A pair, A^T eviction, transpose, reciprocal,
        # normalize) is deferred into the NEXT head's slot 1 so the ACT engine
        # always has the next head's scores queued.
        anorms_all = {
            qb: [
                wrk.tile([P, HID], BF16, tag=f"an{qt}", name=f"an{qb}_{qt}")
                for qt in range(NQT)
            ]
            for qb in range(NQB)
        }

        def make_S(qb, h, t, po, pts):
            def emit_S(g):
                ps = ps_s.tile([P, 2, QBLK], F32, tag="s", name=f"s{qb}_{h}_{g}")
                for half in range(2):
                    kt = 2 * g + half
                    nc.tensor.matmul(
                        ps[:, half, :],
                        lhsT=kt_sb[t][po : po + DHEAD, kt * P : (kt + 1) * P],
                        rhs=qt_sb[t][po : po + DHEAD, qb * QBLK : (qb + 1) * QBLK],
                        start=True,
                        stop=True,
                    )
                pt = ptp.tile([P, 2, QBLK], BF16, tag="pt", name=f"p{qb}_{h}_{g}")
                nc.scalar.activation(pt, ps, EXP)
                nc.vector.tensor_mul(pt, pt, mks[qb][:, 2 * g : 2 * g + 2, :])
                pts[g] = pt

            return emit_S

        def make_A(h, ps_acc, pts):
            def emit_A(g):
                for half in range(2):
                    kt = 2 * g + half
                    nc.tensor.matmul(
                        ps_acc,
                        lhsT=v_sb[kt][:, h, :],
                        rhs=pts[g][:, half, :],
                        start=(g == 0 and half == 0),
                        stop=(g == NG - 1 and half == 1),
                    )

            return emit_A

        def emit_drain(qb, h, emit_A):
            at = atp.tile([VW, QBLK], BF16, tag="at", name=f"at{qb}_{h}")
            nc.gpsimd.tensor_copy(at, ps_accs[(qb, h)])
            if qb < NQB - 1:
                a_t = wrk.tile([P, NQT, VW], BF16, tag=f"a_t{h}", name=f"a_t{qb}_{h}")
                nc.sync.dma_start_transpose(a_t, at)
            else:
                # tail q-block: PE transposes (lower latency than xbar DMA);
                # lives in the ps_o pool, which is idle between V-proj and the
                # output projections.
                a_t = ps_o.tile([P, NQT, VW], BF16, tag="o", name=f"a_tp{qb}_{h}")
                for qt in range(NQT):
                    nc.tensor.transpose(
                        a_t[:, qt, :], at[:, qt * P : (qt + 1) * P], ident[0:VW, 0:VW]
                    )
            rec = wrk.tile([P, NQT, 1], F32, tag=f"rec{h}", name=f"rec{qb}_{h}")
            nc.vector.reciprocal(rec, a_t[:, :, DHEAD : DHEAD + 1])
            # normalize immediately: frees a_t (PSUM in the tail case); in the
            # tail, split across DVE and GPSIMD so the chains run in parallel
            for qt in range(NQT):
                eng = nc.gpsimd if (qb == NQB - 1 and qt % 2 == 1) else nc.vector
                eng.tensor_scalar_mul(
                    anorms_all[qb][qt][:, h * DHEAD : (h + 1) * DHEAD],
                    a_t[:, qt, 0:DHEAD],
                    rec[:, qt, :],
                )

        def emit_qtloop(qb):
            for qt in range(NQT):
                anorm = anorms_all[qb][qt]
                if qb < NQB - 1:
                    att = wrk.tile([P, NCT, P], BF16, tag="att", name=f"att{qb}_{qt}")
                    nc.sync.dma_start_transpose(att, anorm)
                else:
                    attp = ps_a.tile([P, NCT, P], BF16, tag="a", name=f"attp{qb}_{qt}")
                    for ct in range(NCT):
                        nc.tensor.transpose(
                            attp[:, ct, :], anorm[:, ct * P : (ct + 1) * P], ident
                        )
                    att = wrk.tile([P, NCT, P], BF16, tag="att", name=f"att{qb}_{qt}")
                    nc.vector.tensor_copy(att, attp)
                ps_out = ps_o.tile([P, HID], F32, tag="o", name=f"o{qb}_{qt}")
                for ct in range(NCT):
                    nc.tensor.matmul(
                        ps_out,
                        lhsT=att[:, ct, :],
                        rhs=wo_sb[ct],
                        start=(ct == 0),
                        stop=(ct == NCT - 1),
                    )
                ob = obp.tile([P, HID], F32, tag="ob", name=f"ob{qb}_{qt}")
                nc.gpsimd.tensor_copy(ob, ps_out)
                q0 = qb * QBLK + qt * P
                nc.sync.dma_start(out=out_d[q0 : q0 + P, :], in_=ob)

        # A-matmuls trail their S-group by an adaptive slot count (deep early,
        # while the mask DMAs are still streaming in; shallow later so the
        # final drain is short).  Each popped A that closes a head's
        # accumulation immediately triggers that head's drain.
        ps_accs = {}
        pending = []  # FIFO of (qb, h, g, emit_A, is_last_of_head)

        def pop_A():
            qb_, h_, g_, eA, last = pending.pop(0)
            eA(g_)
            if last:
                emit_drain(qb_, h_, eA)
                if h_ == NHEAD - 1:
                    emit_qtloop(qb_)

        for qb in range(NQB):
            for h in range(NHEAD):
                t, po = h // 2, (h % 2) * DHEAD
                ps_accs[(qb, h)] = ps_a.tile([VW, QBLK], F32, tag="a", name=f"a{qb}_{h}")
                pts = [None] * NG
                emit_S = make_S(qb, h, t, po, pts)
                emit_A = make_A(h, ps_accs[(qb, h)], pts)
                depth = 4 if (qb == 0 and h < 2) else 2
                for g in range(NG):
                    emit_S(g)
                    if qb == 0 and h == 0:
                        for kt in V_SLOT[g]:
                            vproj(kt)
                        if g in K_SLOT_H0:
                            kproj_chunk(*K_SLOT_H0[g])
                    if qb == 0 and h == 1 and g in K_SLOT_H1:
                        kproj_chunk(*K_SLOT_H1[g])
                    if qb == 0 and h == 3 and g == 3:
                        qproj_chunk(0, 1)
                    if qb == 0 and h == 3 and g == 5:
                        qproj_chunk(1, 1)
                    pending.append((qb, h, g, emit_A, g == NG - 1))
                    while len(pending) > depth:
                        pop_A()
        while pending:
            pop_A()
    nc.compile()
    return nc


_NC_CACHE = {}
_last_in_maps = None


def _get_nc() -> bass.Bass:
    if "nc" not in _NC_CACHE:
        _NC_CACHE["nc"] = build()
    return _NC_CACHE["nc"]


def kernel(q_hidden_states, k_hidden_states, attention_mask, align_mask, Wq, Wk, Wv, Wo):
    from concourse.bass_utils import run_bass_kernel_spmd

    q_hidden_states = np.asarray(q_hidden_states, np.float32)
    k_hidden_states = np.asarray(k_hidden_states, np.float32)
    attention_mask = np.asarray(attention_mask, np.float32)
    align_mask = np.asarray(align_mask)
    B, Q, _ = q_hidden_states.shape
    qh_len = Q // 2  # 1024

    nc = _get_nc()

    wq = (np.ascontiguousarray(np.asarray(Wq, np.float32).T) / np.float32(8.0)).astype(ml_dtypes.bfloat16)
    wk = np.ascontiguousarray(np.asarray(Wk, np.float32).T).astype(ml_dtypes.bfloat16)
    wv = np.ascontiguousarray(np.asarray(Wv, np.float32).T).astype(ml_dtypes.bfloat16)
    wo = np.ascontiguousarray(np.asarray(Wo, np.float32).T).astype(ml_dtypes.bfloat16)

    use_mask = bool(np.any(attention_mask))

    in_maps = []
    for core in range(8):
        b, qh = divmod(core, 2)
        qsl = slice(qh * qh_len, (qh + 1) * qh_len)
        # multiplicative mask: align * exp(attention_mask)  (exact: the
        # reference adds attention_mask pre-exp and zeroes where align==0)
        mmask = align_mask[b, :, qsl].astype(np.float32)
        if use_mask:
            mmask = mmask * np.exp(
                np.ascontiguousarray(attention_mask[b, 0, qsl, :].T), dtype=np.float32
            )
        m = {
            "xqT": np.ascontiguousarray(q_hidden_states[b, qsl].T).astype(ml_dtypes.bfloat16),
            "xkT": np.ascontiguousarray(k_hidden_states[b].T).astype(ml_dtypes.bfloat16),
            "mm": np.ascontiguousarray(mmask.astype(ml_dtypes.bfloat16)),
            "wqT": wq,
            "wkT": wk,
            "wvT": wv,
            "woT": wo,
        }
        in_maps.append(m)

    global _last_in_maps
    _last_in_maps = in_maps
    res = run_bass_kernel_spmd(nc, in_maps, list(range(8))).results
    out = np.empty((B, Q, HID), np.float32)
    for core in range(8):
        b, qh = divmod(core, 2)
        out[b, qh * qh_len : (qh + 1) * qh_len] = res[core]["out"]
    return out


# revision 19
# speedup vs baseline: 1.0495x; 1.0096x over previous
"""KgAdapterCrossAttention kernel for 8 trn2 NeuronCores.

Sharding: core = (batch b, query-half qh).  Each core computes attention for
1024 queries of one batch element against all 2048 keys.

Engine plan (per core):
  - PE: projections in float32r (1 cyc/row at ap>=256), attention matmuls in
    bf16.  Scores computed transposed S^T [k, q] per head; A computed as
    A^T = V''^T P^T with ap_size=512 and a ones-column in V'' providing the
    softmax denominator (padded to 80 cols so transposes are xbar-friendly).
  - ACT: exp over 2-bank PSUM tiles [128, 2, 512] writing bf16 P^T to SBUF.
    ACT is the bottleneck engine (~66us of exp); everything else is
    scheduled to keep it saturated from ~6us to the end.
  - DVE: multiplicative align-mask (bf16, 2x mode), projection evictions,
    per-q normalize.
  - GPSIMD: PSUM evictions (V'', A^T, output tiles).
  - DMA xbar: transposes (A^T -> A, anorm -> att) for the first q-block;
    the last q-block uses PE transposes to shorten the drain tail.
  - Startup: need-ordered chunked DMAs; V-proj and the K-proj tail are
    interleaved into the first head's pipeline slots.
  - softmax without max-subtraction (scores ~N(0,1)); attention_mask folded
    into the multiplicative mask on host: align * exp(attn_mask) (exact).
"""

import os
import sys

import numpy as np

try:
    import concourse.bass as bass
except ImportError:
    for _p in ("/opt/trn_rl_repo", os.path.expanduser("~/.axon_site/_ro/trn_rl_repo")):
        if os.path.isdir(_p) and _p not in sys.path:
            sys.path.insert(0, _p)
    import concourse.bass as bass

import ml_dtypes
import concourse.mybir as mybir
import concourse.tile as tile
from concourse import bacc
from concourse.masks import make_identity
from contextlib import ExitStack

F32 = mybir.dt.float32
F32R = mybir.dt.float32r
BF16 = mybir.dt.bfloat16
EXP = mybir.ActivationFunctionType.Exp

P = 128
HID = 256
NHEAD = 4
DHEAD = 64
NQ = 1024  # queries per core
NK = 2048  # keys (full)
QBLK = 512
NQB = NQ // QBLK  # 2
NKT = NK // P  # 16
NG = NKT // 2  # 8 kt-pair groups per (qb, h)
NCT = HID // P  # 2 contraction tiles over hidden
VW = 80  # A^T width: 64 V cols + 1 ones col + 15 zero pad (xbar wants %16)
NQT = QBLK // P  # 4


def build() -> bass.Bass:
    nc = bacc.Bacc()
    xqT = nc.declare_dram_parameter("xqT", [HID, NQ], BF16, isOutput=False)
    xkT = nc.declare_dram_parameter("xkT", [HID, NK], BF16, isOutput=False)
    mm = nc.declare_dram_parameter("mm", [NK, NQ], BF16, isOutput=False)
    wqT = nc.declare_dram_parameter("wqT", [HID, HID], BF16, isOutput=False)
    wkT = nc.declare_dram_parameter("wkT", [HID, HID], BF16, isOutput=False)
    wvT = nc.declare_dram_parameter("wvT", [HID, HID], BF16, isOutput=False)
    woT = nc.declare_dram_parameter("woT", [HID, HID], BF16, isOutput=False)
    out_d = nc.declare_dram_parameter("out", [NQ, HID], F32, isOutput=True)

    def R(ap):
        return ap.bitcast(F32R)

    with tile.TileContext(nc) as tc, ExitStack() as ctx:
        const = ctx.enter_context(tc.tile_pool(name="const", bufs=1))
        big = ctx.enter_context(tc.tile_pool(name="big", bufs=1))
        ptp = ctx.enter_context(tc.tile_pool(name="ptp", bufs=14))
        atp = ctx.enter_context(tc.tile_pool(name="atp", bufs=2))
        wrk = ctx.enter_context(tc.tile_pool(name="wrk", bufs=2))
        obp = ctx.enter_context(tc.tile_pool(name="obp", bufs=2))
        ps_s = ctx.enter_context(tc.tile_pool(name="ps_s", bufs=2, space="PSUM"))
        ps_a = ctx.enter_context(tc.tile_pool(name="ps_a", bufs=2, space="PSUM"))
        ps_o = ctx.enter_context(tc.tile_pool(name="ps_o", bufs=2, space="PSUM"))

        # ---- DMA emission, ordered by first need ----
        def loadw(name, src, dt=BF16, cols=None):
            ts = []
            for t in range(2):
                tl = const.tile([P, HID], dt, tag=f"{name}{t}", name=f"{name}{t}")
                nc.sync.dma_start(out=tl, in_=src[t * P : (t + 1) * P, :])
                ts.append(tl)
            return ts

        wq_sb = loadw("wq", wqT)
        wk_sb = loadw("wk", wkT)
        xk_sb = [big.tile([P, NK], BF16, tag=f"xk{t}", name=f"xk{t}") for t in range(2)]
        for t in range(2):  # kt 0..3 — K-proj has the longest startup chain
            nc.sync.dma_start(out=xk_sb[t][:, 0:QBLK], in_=xkT[t * P : (t + 1) * P, 0:QBLK])
        xq_sb = [big.tile([P, NQ], BF16, tag=f"xq{t}", name=f"xq{t}") for t in range(2)]
        for t in range(2):  # first q-block's columns
            nc.sync.dma_start(out=xq_sb[t][:, 0:QBLK], in_=xqT[t * P : (t + 1) * P, 0:QBLK])
        for t in range(2):  # kt 4..7
            nc.sync.dma_start(
                out=xk_sb[t][:, QBLK : 2 * QBLK],
                in_=xkT[t * P : (t + 1) * P, QBLK : 2 * QBLK],
            )

        mm_r = mm.rearrange("(t p) q -> p t q", p=P)
        wv_sb = loadw("wv", wvT)
        for t in range(2):  # chunk b: kt 8..15
            nc.sync.dma_start(
                out=xk_sb[t][:, 1024:2048], in_=xkT[t * P : (t + 1) * P, 1024:2048]
            )
        mk0 = big.tile([P, NKT, QBLK], BF16, tag="mk0", name="mk0")
        for c in range(4):
            nc.sync.dma_start(
                out=mk0[:, 4 * c : 4 * (c + 1), :],
                in_=mm_r[:, 4 * c : 4 * (c + 1), 0:QBLK],
            )
        for t in range(2):  # xq chunk b (only needed by the second q-block)
            nc.sync.dma_start(
                out=xq_sb[t][:, QBLK:NQ], in_=xqT[t * P : (t + 1) * P, QBLK:NQ]
            )
        wo_sb = loadw("wo", woT)
        mk1 = big.tile([P, NKT, QBLK], BF16, tag="mk1", name="mk1")
        nc.sync.dma_start(out=mk1, in_=mm_r[:, :, QBLK:NQ])
        mks = [mk0, mk1]

        # identity (bf16) for PE transposes + ACT exp-table preload
        ident = const.tile([P, P], BF16, tag="ident", name="ident")
        make_identity(nc, ident)
        warm = wrk.tile([P, 1], F32, tag="warm", name="warm")
        nc.gpsimd.memset(warm, 0.0)
        nc.scalar.activation(warm, warm, EXP)

        qt_sb = [big.tile([P, NQ], BF16, tag=f"qt{t}", name=f"qt{t}") for t in range(2)]
        kt_sb = [big.tile([P, NK], BF16, tag=f"kt{t}", name=f"kt{t}") for t in range(2)]
        v_sb = [None] * NKT

        # ---- compute helpers ----
        def qproj_chunk(t, c):  # q columns c*QBLK:(c+1)*QBLK
            ps = ps_s.tile([P, 2, QBLK], F32, tag="s", name=f"qproj{t}_{c}")
            qs = slice(c * QBLK, (c + 1) * QBLK)
            for ct in range(NCT):
                nc.tensor.matmul(
                    ps[:, 0, :],
                    lhsT=wq_sb[ct][:, t * P : (t + 1) * P],
                    rhs=xq_sb[ct][:, qs],
                    start=(ct == 0),
                    stop=(ct == NCT - 1),
                )
            nc.vector.tensor_copy(qt_sb[t][:, qs], ps[:, 0, :])

        def kproj_chunk(t, c):  # k columns c*1024:(c+1)*1024
            ps = ps_s.tile([P, 2, QBLK], F32, tag="s", name=f"kproj{t}_{c}")
            for nb in range(2):
                off = c * 1024 + nb * QBLK
                for ct in range(NCT):
                    nc.tensor.matmul(
                        ps[:, nb, :],
                        lhsT=wk_sb[ct][:, t * P : (t + 1) * P],
                        rhs=xk_sb[ct][:, off : off + QBLK],
                        start=(ct == 0),
                        stop=(ct == NCT - 1),
                    )
            nc.vector.tensor_copy(
                kt_sb[t][:, c * 1024 : (c + 1) * 1024],
                ps.rearrange("p a b -> p (a b)"),
            )

        def vproj(kt):
            ps = ps_o.tile([P, HID], F32, tag="o", name=f"vproj{kt}")
            for ct in range(NCT):
                nc.tensor.matmul(
                    ps,
                    lhsT=xk_sb[ct][:, kt * P : (kt + 1) * P],
                    rhs=wv_sb[ct],
                    start=(ct == 0),
                    stop=(ct == NCT - 1),
                )
            tl = big.tile([P, NHEAD, VW], BF16, tag=f"v{kt}", name=f"v{kt}")
            nc.vector.tensor_copy(
                tl[:, :, 0:DHEAD], ps.rearrange("p (h d) -> p h d", h=NHEAD)
            )
            nc.gpsimd.memset(tl[:, :, DHEAD : DHEAD + 1], 1.0)
            nc.gpsimd.memset(tl[:, :, DHEAD + 1 : VW], 0.0)
            v_sb[kt] = tl

        # Startup: K cols 0:512 of t0 first (longest chain to the first
        # S-matmul), then Q chunk a, then K cols 512:1024 — each 512-col piece
        # in its own psum tile so the first S-group only waits on what it reads.
        def k512(c):
            ps = ps_s.tile([P, 2, QBLK], F32, tag="s", name=f"kproj_a{c}")
            for ct in range(NCT):
                nc.tensor.matmul(
                    ps[:, 0, :],
                    lhsT=wk_sb[ct][:, 0:P],
                    rhs=xk_sb[ct][:, c * QBLK : (c + 1) * QBLK],
                    start=(ct == 0),
                    stop=(ct == NCT - 1),
                )
            nc.vector.tensor_copy(kt_sb[0][:, c * QBLK : (c + 1) * QBLK], ps[:, 0, :])

        k512(0)
        ps_q = ps_s.tile([P, 2, QBLK], F32, tag="s", name="qproj_a")
        for t in range(2):
            for ct in range(NCT):
                nc.tensor.matmul(
                    ps_q[:, t, :],
                    lhsT=wq_sb[ct][:, t * P : (t + 1) * P],
                    rhs=xq_sb[ct][:, 0:QBLK],
                    start=(ct == 0),
                    stop=(ct == NCT - 1),
                )
            nc.vector.tensor_copy(qt_sb[t][:, 0:QBLK], ps_q[:, t, :])
        k512(1)

        # per-slot extra work woven into the first two heads of qb0
        V_SLOT = {g: [2 * g, 2 * g + 1] for g in range(NG)}  # h0 slots
        K_SLOT_H0 = {2: (0, 1)}  # K-proj chunk (t, c) by slot
        K_SLOT_H1 = {0: (1, 0), 1: (1, 1)}

        # ---- attention: one flat slot stream over all (qb, h) pairs ----
        # Each head's drain (last A pair, A^T eviction, transpose, reciprocal,
        # normalize) is deferred into the NEXT head's slot 1 so the ACT engine
        # always has the next head's scores queued.
        anorms_all = {
            qb: [
                wrk.tile([P, HID], BF16, tag=f"an{qt}", name=f"an{qb}_{qt}")
                for qt in range(NQT)
            ]
            for qb in range(NQB)
        }

        def make_S(qb, h, t, po, pts):
            def emit_S(g):
                ps = ps_s.tile([P, 2, QBLK], F32, tag="s", name=f"s{qb}_{h}_{g}")
                for half in range(2):
                    kt = 2 * g + half
                    nc.tensor.matmul(
                        ps[:, half, :],
                        lhsT=kt_sb[t][po : po + DHEAD, kt * P : (kt + 1) * P],
                        rhs=qt_sb[t][po : po + DHEAD, qb * QBLK : (qb + 1) * QBLK],
                        start=True,
                        stop=True,
                    )
                pt = ptp.tile([P, 2, QBLK], BF16, tag="pt", name=f"p{qb}_{h}_{g}")
                nc.scalar.activation(pt, ps, EXP)
                # mask multiply: SBUF-only, so a quarter can run on the
                # otherwise-idle GPSIMD to keep DVE off the critical path
                eng = nc.gpsimd if g in (1, 5) else nc.vector
                eng.tensor_mul(pt, pt, mks[qb][:, 2 * g : 2 * g + 2, :])
                pts[g] = pt

            return emit_S

        def make_A(h, ps_acc, pts):
            def emit_A(g):
                for half in range(2):
                    kt = 2 * g + half
                    nc.tensor.matmul(
                        ps_acc,
                        lhsT=v_sb[kt][:, h, :],
                        rhs=pts[g][:, half, :],
                        start=(g == 0 and half == 0),
                        stop=(g == NG - 1 and half == 1),
                    )

            return emit_A

        def emit_drain(qb, h, emit_A):
            at = atp.tile([VW, QBLK], BF16, tag="at", name=f"at{qb}_{h}")
            nc.vector.tensor_copy(at, ps_accs[(qb, h)])
            if qb < NQB - 1:
                a_t = wrk.tile([P, NQT, VW], BF16, tag=f"a_t{h}", name=f"a_t{qb}_{h}")
                nc.sync.dma_start_transpose(a_t, at)
            else:
                # tail q-block: PE transposes (lower latency than xbar DMA);
                # lives in the ps_o pool, which is idle between V-proj and the
                # output projections.
                a_t = ps_o.tile([P, NQT, VW], BF16, tag="o", name=f"a_tp{qb}_{h}")
                for qt in range(NQT):
                    nc.tensor.transpose(
                        a_t[:, qt, :], at[:, qt * P : (qt + 1) * P], ident[0:VW, 0:VW]
                    )
            rec = wrk.tile([P, NQT, 1], F32, tag=f"rec{h}", name=f"rec{qb}_{h}")
            nc.vector.reciprocal(rec, a_t[:, :, DHEAD : DHEAD + 1])
            # normalize immediately: frees a_t (PSUM in the tail case)
            for qt in range(NQT):
                nc.vector.tensor_scalar_mul(
                    anorms_all[qb][qt][:, h * DHEAD : (h + 1) * DHEAD],
                    a_t[:, qt, 0:DHEAD],
                    rec[:, qt, :],
                )

        def emit_qtloop(qb):
            for qt in range(NQT):
                anorm = anorms_all[qb][qt]
                if qb < NQB - 1:
                    att = wrk.tile([P, NCT, P], BF16, tag="att", name=f"att{qb}_{qt}")
                    nc.sync.dma_start_transpose(att, anorm)
                else:
                    attp = ps_a.tile([P, NCT, P], BF16, tag="a", name=f"attp{qb}_{qt}")
                    for ct in range(NCT):
                        nc.tensor.transpose(
                            attp[:, ct, :], anorm[:, ct * P : (ct + 1) * P], ident
                        )
                    att = wrk.tile([P, NCT, P], BF16, tag="att", name=f"att{qb}_{qt}")
                    nc.vector.tensor_copy(att, attp)
                ps_out = ps_o.tile([P, HID], F32, tag="o", name=f"o{qb}_{qt}")
                for ct in range(NCT):
                    nc.tensor.matmul(
                        ps_out,
                        lhsT=att[:, ct, :],
                        rhs=wo_sb[ct],
                        start=(ct == 0),
                        stop=(ct == NCT - 1),
                    )
                ob = obp.tile([P, HID], F32, tag="ob", name=f"ob{qb}_{qt}")
                nc.vector.tensor_copy(ob, ps_out)
                q0 = qb * QBLK + qt * P
                nc.sync.dma_start(out=out_d[q0 : q0 + P, :], in_=ob)

        # A-matmuls trail their S-group by an adaptive slot count (deep early,
        # while the mask DMAs are still streaming in; shallow later so the
        # final drain is short).  Each popped A that closes a head's
        # accumulation immediately triggers that head's drain.
        ps_accs = {}
        pending = []  # FIFO of (qb, h, g, emit_A, is_last_of_head)

        def pop_A():
            qb_, h_, g_, eA, last = pending.pop(0)
            eA(g_)
            if last:
                emit_drain(qb_, h_, eA)
                if h_ == NHEAD - 1:
                    emit_qtloop(qb_)

        for qb in range(NQB):
            for h in range(NHEAD):
                t, po = h // 2, (h % 2) * DHEAD
                ps_accs[(qb, h)] = ps_a.tile([VW, QBLK], F32, tag="a", name=f"a{qb}_{h}")
                pts = [None] * NG
                emit_S = make_S(qb, h, t, po, pts)
                emit_A = make_A(h, ps_accs[(qb, h)], pts)
                depth = 4 if (qb == 0 and h < 2) else 2
                for g in range(NG):
                    emit_S(g)
                    if qb == 0 and h == 0:
                        for kt in V_SLOT[g]:
                            vproj(kt)
                        if g in K_SLOT_H0:
                            kproj_chunk(*K_SLOT_H0[g])
                    if qb == 0 and h == 1 and g in K_SLOT_H1:
                        kproj_chunk(*K_SLOT_H1[g])
                    if qb == 0 and h == 3 and g == 3:
                        qproj_chunk(0, 1)
                    if qb == 0 and h == 3 and g == 5:
                        qproj_chunk(1, 1)
                    pending.append((qb, h, g, emit_A, g == NG - 1))
                    while len(pending) > depth:
                        pop_A()
        while pending:
            pop_A()
    nc.compile()
    return nc


_NC_CACHE = {}
_last_in_maps = None


def _get_nc() -> bass.Bass:
    if "nc" not in _NC_CACHE:
        _NC_CACHE["nc"] = build()
    return _NC_CACHE["nc"]


def kernel(q_hidden_states, k_hidden_states, attention_mask, align_mask, Wq, Wk, Wv, Wo):
    from concourse.bass_utils import run_bass_kernel_spmd

    q_hidden_states = np.asarray(q_hidden_states, np.float32)
    k_hidden_states = np.asarray(k_hidden_states, np.float32)
    attention_mask = np.asarray(attention_mask, np.float32)
    align_mask = np.asarray(align_mask)
    B, Q, _ = q_hidden_states.shape
    qh_len = Q // 2  # 1024

    nc = _get_nc()

    wq = (np.ascontiguousarray(np.asarray(Wq, np.float32).T) / np.float32(8.0)).astype(ml_dtypes.bfloat16)
    wk = np.ascontiguousarray(np.asarray(Wk, np.float32).T).astype(ml_dtypes.bfloat16)
    wv = np.ascontiguousarray(np.asarray(Wv, np.float32).T).astype(ml_dtypes.bfloat16)
    wo = np.ascontiguousarray(np.asarray(Wo, np.float32).T).astype(ml_dtypes.bfloat16)

    use_mask = bool(np.any(attention_mask))

    in_maps = []
    for core in range(8):
        b, qh = divmod(core, 2)
        qsl = slice(qh * qh_len, (qh + 1) * qh_len)
        # multiplicative mask: align * exp(attention_mask)  (exact: the
        # reference adds attention_mask pre-exp and zeroes where align==0)
        mmask = align_mask[b, :, qsl].astype(np.float32)
        if use_mask:
            mmask = mmask * np.exp(
                np.ascontiguousarray(attention_mask[b, 0, qsl, :].T), dtype=np.float32
            )
        m = {
            "xqT": np.ascontiguousarray(q_hidden_states[b, qsl].T).astype(ml_dtypes.bfloat16),
            "xkT": np.ascontiguousarray(k_hidden_states[b].T).astype(ml_dtypes.bfloat16),
            "mm": np.ascontiguousarray(mmask.astype(ml_dtypes.bfloat16)),
            "wqT": wq,
            "wkT": wk,
            "wvT": wv,
            "woT": wo,
        }
        in_maps.append(m)

    global _last_in_maps
    _last_in_maps = in_maps
    res = run_bass_kernel_spmd(nc, in_maps, list(range(8))).results
    out = np.empty((B, Q, HID), np.float32)
    for core in range(8):
        b, qh = divmod(core, 2)
        out[b, qh * qh_len : (qh + 1) * qh_len] = res[core]["out"]
    return out
